# revision 5
# baseline (speedup 1.0000x reference)
"""Causal self-attention on 8 TRN2 NeuronCores.

Sharding: batch (2) x head-groups (4 heads each) -> 8 cores. Each core
computes the qkv projection for its 4 heads, causal attention over
lower-triangular 128-blocks, and a partial o-projection. Host transposes the
per-head attention maps (device emits them j-major), sums o partials across
head groups, and adds the output bias.

Scores are computed only in transposed layout [j, i]: softmax row-sums come
from a ones-column appended to V (matmul row 64), 1/sum is broadcast across
partitions with gpsimd, and normalization is a tensor-tensor multiply. This
keeps the PE stream pure fp32r matmuls (no transpose-mode ops that confuse
the HAM clock gate) and each attention element passes through ACT exactly
once (the exp) plus one DVE multiply.

Self-contained: hardcodes B=2, T=2048, C=1024, H=16, D=64.
"""

import numpy as np
from contextlib import ExitStack

import concourse.bass as bass
import concourse.tile as tile
import concourse.mybir as mybir
from concourse import bacc
import concourse.bass_utils as bass_utils

F32 = mybir.dt.float32
F32R = mybir.dt.float32r
AF = mybir.ActivationFunctionType
ALU = mybir.AluOpType

P = 128
T = 2048
C = 1024
D = 64
HL = 4          # local heads per core
NQKV = 3 * HL * D   # 768
TB = T // P     # 16 row blocks
CB = C // P     # 8 contraction chunks for qkv
MB = NQKV // P  # 6 qkv output row-blocks
TCH = T // 512  # 4 column chunks of 512
NEG = -1e30


def build_nc():
    nc = bacc.Bacc("TRN2", target_bir_lowering=False, debug=False)

    x = nc.dram_tensor("x", [T, C], F32, kind="ExternalInput").ap()
    w3 = nc.dram_tensor("w3", [C, NQKV], F32, kind="ExternalInput").ap()
    b3 = nc.dram_tensor("b3", [NQKV], F32, kind="ExternalInput").ap()
    wo = nc.dram_tensor("wo", [HL * D, C], F32, kind="ExternalInput").ap()
    ident_in = nc.dram_tensor("ident", [P, P], F32, kind="ExternalInput").ap()
    id2_in = nc.dram_tensor("id2", [P, D], F32, kind="ExternalInput").ap()
    maskT_in = nc.dram_tensor("maskT", [P, P], F32, kind="ExternalInput").ap()
    ones4_in = nc.dram_tensor("ones4", [P, HL], F32, kind="ExternalInput").ap()

    # attention, transposed per head: attn4T[h, j, i] = attn_w[h, i, j]
    attn4T = nc.dram_tensor("attn4T", [HL, T, T], F32, kind="ExternalOutput").ap()
    o_part = nc.dram_tensor("o_part", [T, C], F32, kind="ExternalOutput").ap()

    with ExitStack() as ctx:
        tc = ctx.enter_context(tile.TileContext(nc))
        const = ctx.enter_context(tc.tile_pool(name="const", bufs=1))
        persist = ctx.enter_context(tc.tile_pool(name="persist", bufs=1))
        srow = ctx.enter_context(tc.tile_pool(name="srow", bufs=2))
        pool_mm = ctx.enter_context(tc.tile_pool(name="pmm", bufs=3, space="PSUM"))
        pool_tr = ctx.enter_context(tc.tile_pool(name="ptr", bufs=2, space="PSUM"))
        pool_av = ctx.enter_context(tc.tile_pool(name="pav", bufs=2, space="PSUM"))

        # ---- constants ----
        ident = const.tile([P, P], F32)
        nc.sync.dma_start(ident[:], ident_in)
        id2 = const.tile([P, D], F32)
        nc.sync.dma_start(id2[:], id2_in)
        id2_r = const.tile([P, D], F32R)
        nc.vector.tensor_copy(id2_r[:], id2[:])
        maskT = const.tile([P, P], F32)
        nc.sync.dma_start(maskT[:], maskT_in)
        ones4 = const.tile([P, HL], F32)
        nc.sync.dma_start(ones4[:], ones4_in)
        ones4_r = const.tile([P, HL], F32R)
        nc.vector.tensor_copy(ones4_r[:], ones4[:])

        b_sb = const.tile([P, MB], F32)
        nc.sync.dma_start(b_sb[:], b3.rearrange("(o p) -> p o", p=P))

        # rounded weights (fp32r matmul operands must be compute-produced)
        wo_r4 = persist.tile([D, HL, C], F32R)
        qkT = persist.tile([P, 4, T], F32R)     # [q0q1 | q2q3 | k0k1 | k2k3]
        v1 = persist.tile([P, TB, HL, D + 1], F32R)  # v natural + ones col

        pool_w = ctx.enter_context(tc.tile_pool(name="pw", bufs=1))
        pool_vt = ctx.enter_context(tc.tile_pool(name="pvt", bufs=1))
        w_r = pool_w.tile([P, CB, NQKV], F32R)
        vT_tmp = pool_vt.tile([P, 2, T], F32R)
        with tc.tile_pool(name="wtmp", bufs=2) as wtmp:
            for cb in range(CB):
                t_ = wtmp.tile([P, NQKV], F32, tag="wld")
                nc.sync.dma_start(t_[:], w3[cb * P:(cb + 1) * P, :])
                nc.any.tensor_copy(w_r[:, cb, :], t_[:])
            for h in range(HL):
                t_ = wtmp.tile([D, C], F32, tag="wold")
                nc.sync.dma_start(t_[:], wo[h * D:(h + 1) * D, :])
                nc.any.tensor_copy(wo_r4[:, h, :], t_[:])

        # ---- phase AB: per 512-col chunk: transpose x, project to qkvT ----
        def qkv_dst(mb):
            # mb 0..3 -> qkT rows, mb 4..5 -> scoped vT_tmp
            return qkT[:, mb, :] if mb < 4 else vT_tmp[:, mb - 4, :]

        with (
            tc.tile_pool(name="xld", bufs=3) as pool_x,
            tc.tile_pool(name="xtc", bufs=2) as pool_xt,
        ):
            for tch in range(TCH):
                xT_chunk = pool_xt.tile([P, CB, 512], F32R)
                for tbl in range(4):
                    tb = tch * 4 + tbl
                    x_tb = pool_x.tile([P, C], F32)
                    nc.sync.dma_start(x_tb[:], x[tb * P:(tb + 1) * P, :])
                    for cbp in range(2):   # pack 4 transposes per psum bank
                        pst = pool_tr.tile([P, 512], F32, tag="tr")
                        for k in range(4):
                            cb = cbp * 4 + k
                            nc.tensor.transpose(
                                pst[:, k * P:(k + 1) * P],
                                x_tb[:, cb * P:(cb + 1) * P],
                                ident[:],
                            )
                        for k in range(4):
                            cb = cbp * 4 + k
                            nc.any.tensor_copy(
                                xT_chunk[:, cb, tbl * P:(tbl + 1) * P],
                                pst[:, k * P:(k + 1) * P],
                            )
                for mb in range(MB):
                    psq = pool_mm.tile([P, 512], F32, tag="mm")
                    for cb in range(CB):
                        nc.tensor.matmul(
                            psq[:],
                            w_r[:, cb, mb * P:(mb + 1) * P],
                            xT_chunk[:, cb, :],
                            start=(cb == 0),
                            stop=(cb == CB - 1),
                        )
                    nc.scalar.activation(
                        qkv_dst(mb)[:, tch * 512:(tch + 1) * 512], psq[:],
                        AF.Identity, bias=b_sb[:, mb:mb + 1], scale=1.0,
                    )

        # ---- phase V: v rows -> natural layout + ones column ----
        for h in range(HL):
            base = D * (h % 2)
            vT_h = vT_tmp[base:base + D, h // 2, :]
            for jbp in range(2):    # pack 8 transposes per psum bank
                psv = pool_tr.tile([P, 512], F32R, tag="tr")
                for k in range(8):
                    jb = jbp * 8 + k
                    nc.tensor.transpose(
                        psv[:, k * D:(k + 1) * D],
                        vT_h[:, jb * P:(jb + 1) * P],
                        id2_r[base:base + D, :],
                        tile_position=(base, 0),
                    )
                for k in range(8):
                    jb = jbp * 8 + k
                    nc.any.tensor_copy(
                        v1[:, jb, h, 0:D],
                        psv[:, k * D:(k + 1) * D],
                    )
        for jb in range(TB):
            nc.any.tensor_copy(v1[:, jb, :, D], ones4_r[:])

        # ---- phase C: attention per head, transposed layout ----
        avT = persist.tile([D, HL, T], F32R)
        with tc.tile_pool(name="eTu", bufs=20) as pool_eT, \
             tc.tile_pool(name="etn", bufs=3) as pool_etn:
            for h in range(HL):
                base = D * (h % 2)
                qT_h = qkT[base:base + D, h // 2, :]
                kT_h = qkT[base:base + D, 2 + h // 2, :]
                for c in range(4):
                    njb = 4 * c + 4
                    psav = pool_av.tile([D + 1, 512], F32, tag="av")
                    eTus = []
                    for jb in range(njb):
                        pss = pool_mm.tile([P, 512], F32, tag="mm")
                        nc.tensor.matmul(
                            pss[:],
                            kT_h[:, jb * P:(jb + 1) * P],
                            qT_h[:, c * 512:(c + 1) * 512],
                            start=True, stop=True,
                        )
                        p = jb - 4 * c
                        if p >= 0:
                            if p > 0:
                                nc.vector.tensor_scalar_add(
                                    pss[:, :p * P], pss[:, :p * P], NEG
                                )
                            nc.vector.tensor_tensor(
                                pss[:, p * P:(p + 1) * P],
                                pss[:, p * P:(p + 1) * P],
                                maskT[:], ALU.add,
                            )
                        eTu = pool_eT.tile([P, 512], F32R, tag="eTu")
                        nc.scalar.activation(
                            eTu[:], pss[:], AF.Exp, scale=0.125
                        )
                        nc.tensor.matmul(
                            psav[:],
                            v1[:, jb, h, :],
                            eTu[:],
                            start=(jb == 0), stop=(jb == njb - 1),
                        )
                        eTus.append(eTu)
                    # sums (row 64 of psav) -> broadcast 1/sum
                    sums_sb = srow.tile([D + 1, 512], F32, tag="sums")
                    nc.vector.tensor_copy(sums_sb[D:D + 1, :], psav[D:D + 1, :])
                    row0 = srow.tile([1, 512], F32, tag="row0")
                    nc.sync.dma_start(row0[:], sums_sb[D:D + 1, :])
                    rb = srow.tile([P, 512], F32, tag="rb")
                    nc.gpsimd.partition_broadcast(rb[:], row0[:])
                    nc.vector.reciprocal(rb[:], rb[:])
                    # normalized attn_v (deferred softmax denominator)
                    nc.any.tensor_tensor(
                        avT[:, h, c * 512:(c + 1) * 512],
                        psav[0:D, :], rb[0:D, :], ALU.mult,
                    )
                    # normalize + emit attention tiles (transposed)
                    for jb in range(njb):
                        etn = pool_etn.tile([P, 512], F32, tag="etn")
                        nc.any.tensor_tensor(
                            etn[:], eTus[jb][:], rb[:], ALU.mult
                        )
                        nc.sync.dma_start(
                            attn4T[h, jb * P:(jb + 1) * P,
                                   c * 512:(c + 1) * 512],
                            etn[:],
                        )

        # ---- phase D: o projection (partial; host adds bias + reduces) ----
        with tc.tile_pool(name="osb", bufs=3) as pool_o:
            for tb in range(TB):
                for ncol in range(2):
                    pso = pool_mm.tile([P, 512], F32, tag="mm")
                    for h in range(HL):
                        nc.tensor.matmul(
                            pso[:],
                            avT[:, h, tb * P:(tb + 1) * P],
                            wo_r4[:, h, ncol * 512:(ncol + 1) * 512],
                            start=(h == 0), stop=(h == HL - 1),
                        )
                    o_sb = pool_o.tile([P, 512], F32, tag="osb")
                    nc.any.tensor_copy(o_sb[:], pso[:])
                    nc.sync.dma_start(
                        o_part[tb * P:(tb + 1) * P, ncol * 512:(ncol + 1) * 512],
                        o_sb[:],
                    )

    nc.compile()
    return nc


_NC_CACHE = []


def _get_nc():
    if not _NC_CACHE:
        _NC_CACHE.append(build_nc())
    return _NC_CACHE[0]


def _host_consts():
    ident = np.eye(P, dtype=np.float32)
    id2 = np.concatenate([np.eye(D, dtype=np.float32)] * 2, axis=0)
    maskT = np.tril(np.full((P, P), NEG, dtype=np.float32), -1)
    ones4 = np.ones((P, HL), dtype=np.float32)
    return ident, id2, maskT, ones4


def kernel(x, w_qkv, b_qkv, w_o, b_o, _trace=False, _trace_kwargs=None):
    x = np.ascontiguousarray(np.asarray(x, dtype=np.float32))
    w_qkv = np.asarray(w_qkv, dtype=np.float32)
    b_qkv = np.asarray(b_qkv, dtype=np.float32)
    w_o = np.asarray(w_o, dtype=np.float32)
    b_o = np.asarray(b_o, dtype=np.float32)

    H = 16
    ident, id2, maskT, ones4 = _host_consts()
    in_maps = []
    for core in range(8):
        b = core // 4
        hg = (core % 4) * HL
        cols = np.r_[hg * D:(hg + HL) * D]
        w3 = np.concatenate(
            [w_qkv[:, cols], w_qkv[:, C + cols], w_qkv[:, 2 * C + cols]], axis=1
        )
        b3 = np.concatenate(
            [b_qkv[cols], b_qkv[C + cols], b_qkv[2 * C + cols]]
        )
        in_maps.append({
            "x": np.ascontiguousarray(x[b]),
            "w3": np.ascontiguousarray(w3),
            "b3": np.ascontiguousarray(b3),
            "wo": np.ascontiguousarray(w_o[hg * D:(hg + HL) * D, :]),
            "ident": ident,
            "id2": id2,
            "maskT": maskT,
            "ones4": ones4,
        })

    nc = _get_nc()
    kw = {}
    if _trace:
        kw = dict(trace=True, **(_trace_kwargs or {}))
    res = bass_utils.run_bass_kernel_spmd(
        nc, in_maps, core_ids=list(range(8)), **kw
    )

    attn_w = np.empty((2, H, T, T), dtype=np.float32)
    o = np.zeros((2, T, C), dtype=np.float32)
    for core in range(8):
        b = core // 4
        hg = (core % 4) * HL
        r = res.results[core]
        for h in range(HL):
            attn_w[b, hg + h] = r["attn4T"][h].T
        o[b] += r["o_part"]
    o += b_o

    if _trace:
        return (o, attn_w), res
    return o, attn_w
